# revision 18
# baseline (speedup 1.0000x reference)
"""Trainium2 Bass kernel for nn_Blur1: 3x3 cross blur + LIF neuron scan.

Reference semantics (per timestep t, state v/i per pixel):
    c    = conv2d_same(x[t], K)        # K = cross kernel (0.15 sides, 0.4 ctr)
    v_d  = 0.8*v + 0.2*i
    z[t] = (v_d - 1) > 0
    v    = (1-z)*v_d
    i    = 0.8*i + c

Strategy (8 NeuronCores = 4 H-shards x 2 W-shards, no collectives; halos are
baked into the per-core input slices on the host):
  * Scaled variables remove all per-step scalar multiplies except 0.8:
      I' = i/s_i, V' = v/(0.2*s_i), with s_i = K_left (0.15).
      c' = c/s_i = u + d + l + r + (8/3)x  (for the given cross kernel)
      V'_dec = 0.8 V' + I';  z = V'_dec > TH (TH = 1/(0.2*s_i));
      V' = (V'_dec <= TH) * V'_dec;  I' = 0.8 I' + c'
    Spike output z is bit-identical (validated in fp32 numpy vs jax ref).
  * Per core: 128 rows on the 128 SBUF partitions, 256 local W cols, T=128.
  * Conv: vertical taps (u + (8/3)c + d) via one fp32 PE matmul with a
    tridiagonal stationary matrix (PE fp32 is exact; fp32r is TF32-like and
    NOT usable). Horizontal taps (l+r) on GPSIMD. H-halo rows added via
    SWDGE DMA-accumulate directly from DRAM. All summed into hsum in SBUF.
  * Synaptic current I': one DVE tensor_tensor_scan per (8w x 128t) slice,
    with a 0.8-multiplier tile whose t=0 slots are 0.0 (per-pixel reset),
    so one scan instruction handles 8 independent pixel recurrences.
  * Membrane V': 127 sequential steps of two scalar_tensor_tensor ops on
    [128, 256]; V_dec overwrites the consumed I' slot in place.
  * Spikes: batched ACT sign -> relu over the stored V_dec values.
"""
import sys

for _p in ("/opt/trn_rl_repo",):
    if _p not in sys.path:
        sys.path.insert(0, _p)

import numpy as np
from concourse import bacc, mybir
import concourse.tile as tile
from concourse.bass_utils import run_bass_kernel_spmd

f32 = mybir.dt.float32

T = 128          # timesteps
RPC = 128        # rows per core (H=512 / 4)
WPC = 256        # cols per core (W=512 / 2)
NWC = 4          # w-chunks per core
WC = WPC // NWC  # 64 cols per chunk
NTH = 4          # t-quarters per chunk DMA
TH_T = T // NTH  # 32
SCAN_W = 16      # w-cols per scan op (F = 16*128 = 2048)
HW_W = 32        # w-cols per hsum tile
ZB = 16          # timesteps per z-output block

_CACHE = {}
_LAST_IN_MAPS = None
TUNE = {"xc_bufs": 3, "tmp_bufs": 2, "ps_bufs": 2}


def _register_const(nc, value, dtype=f32):
    t = nc.alloc_sbuf_tensor(f"const-user-{value}", [128, 1], dtype)
    nc.gpsimd.memset(t.ap(), value)
    nc.const_aps.aps[(dtype, value)] = t.ap()


def _build_cached(s_i, k_up, k_ctr, k_down, k_right):
    key = (s_i, k_up, k_ctr, k_down, k_right)
    if key not in _CACHE:
        _CACHE[key] = _build_with_consts(*key)
    return _CACHE[key]


def _build_with_consts(s_i, k_up, k_ctr, k_down, k_right):
    # activation() with a float bias needs a pre-registered const AP; patch
    # the builder to register -TH right after Bass init.
    TH = 1.0 / (0.2 * s_i)
    nc = bacc.Bacc("TRN2", target_bir_lowering=False, debug=False,
                   num_devices=8)
    _register_const(nc, -TH)
    nc.all_engine_barrier()
    _build_body(nc, s_i, k_up, k_ctr, k_down, k_right)
    if not nc.is_finalized():
        nc.finalize()
    return nc


def _build_body(nc, s_i, k_up, k_ctr, k_down, k_right, ablate=()):
    # identical to _build()'s body after nc creation
    DEC = 0.8
    TH = 1.0 / (0.2 * s_i)

    xm = nc.declare_dram_parameter("xm", [T, RPC, WPC + 2], f32, isOutput=False)
    xh = nc.declare_dram_parameter("xh", [2, WPC + 2, T], f32, isOutput=False)
    wv = nc.declare_dram_parameter("wv", [RPC, RPC], f32, isOutput=False)
    zo = nc.declare_dram_parameter("zo", [T, RPC, WPC], f32, isOutput=True)

    with tile.TileContext(nc) as tc:
        with tc.tile_pool(name="keep", bufs=1) as keep:
            wvt = keep.tile([RPC, RPC], f32)
            nc.sync.dma_start(wvt[:], wv[:])

            It = keep.tile([128, WPC * T], f32)
            Iv = It[:].rearrange("p (w t) -> p w t", t=T)

            Vt = keep.tile([128, WPC], f32)
            nc.vector.memset(Vt[:], 0.0)

            d0 = keep.tile([128, SCAN_W * T], f32)
            nc.vector.memset(d0[:], DEC)
            d0v = d0[:].rearrange("p (w t) -> p w t", t=T)
            nc.vector.memset(d0v[:, :, 0:1], 0.0)

            TBS = [2, 2, 4, 8] + [16] * 7   # t-block sizes (pipeline priming)
            with tc.tile_pool(name="xc", bufs=TUNE["xc_bufs"]) as xcp, \
                 tc.tile_pool(name="tmp", bufs=TUNE["tmp_bufs"]) as tmpp, \
                 tc.tile_pool(name="ps", bufs=TUNE["ps_bufs"], space="PSUM") as psp:
                dma_engines = [nc.sync, nc.scalar]
                # c-prime accumulates directly in the I buffer (Iv views).
                t0 = 0
                for tb, TB in enumerate(TBS):
                    xc = xcp.tile([128, max(TBS) * (WPC + 2)], f32,
                                  tag="xc", name="xc")[:, :TB * (WPC + 2)]
                    xcv = xc.rearrange("p (t w) -> p t w", w=WPC + 2)
                    dma_engines[tb % 2].dma_start(
                        xcv,
                        xm[t0:t0 + TB, :, :].rearrange("t p w -> p t w"))

                    for s in range(8 if "hsum" not in ablate else 0):
                        ws = s * 32   # local w of this 32-col slice
                        nc.gpsimd.tensor_tensor(
                            Iv[:, ws:ws + 32, t0:t0 + TB],
                            xcv[:, :, ws:ws + 32].rearrange("p t w -> p w t"),
                            xcv[:, :, ws + 2:ws + 34].rearrange("p t w -> p w t"),
                            mybir.AluOpType.add)

                    for q in range(2 if "pe" not in ablate else 0):
                        wq = q * 128
                        pst = psp.tile([128, 2048], f32, tag="pst", name="pst")
                        for m in range(4):
                            wg = wq + m * 32
                            nc.tensor.matmul(
                                pst[:, m * 512:m * 512 + 32 * TB],
                                wvt[:],
                                xcv[:, :, 1 + wg:33 + wg]
                                   .rearrange("p t w -> p w t"),
                                start=True, stop=True)
                        nc.vector.tensor_tensor(
                            Iv[:, wq:wq + 128, t0:t0 + TB],
                            Iv[:, wq:wq + 128, t0:t0 + TB],
                            pst[:].rearrange("p (m c) -> p m c", m=4)
                                [:, :, :32 * TB]
                                .rearrange("p m (w t) -> p m w t", t=TB),
                            mybir.AluOpType.add)
                    t0 += TB

                # H-halo rows into partitions 0 / 127, then scan + copy-back
                for s in range(8):
                    ws = s * 32
                    nc.gpsimd.dma_start(
                        Iv[0:1, ws:ws + 32, :], xh[0:1, 1 + ws:1 + ws + 32, :],
                        accum_op=mybir.AluOpType.add)
                    nc.gpsimd.dma_start(
                        Iv[127:128, ws:ws + 32, :], xh[1:2, 1 + ws:1 + ws + 32, :],
                        accum_op=mybir.AluOpType.add)
                    for k in range(2 if "scan" not in ablate else 0):
                        lo = (ws + k * SCAN_W) * T
                        hi = (ws + (k + 1) * SCAN_W) * T
                        tmp = tmpp.tile([128, SCAN_W * T], f32,
                                        tag="tmp", name="tmp")
                        nc.vector.tensor_tensor_scan(
                            tmp[:], d0[:], It[:, lo:hi],
                            0.0, mybir.AluOpType.mult, mybir.AluOpType.add)
                        nc.scalar.copy(It[:, lo:hi], tmp[:])

            with tc.tile_pool(name="zs", bufs=2) as zsp, \
                 tc.tile_pool(name="sc", bufs=2) as scp:
                z_ends = [16, 32, 48, 64, 80, 96, 112, 120, 124, 126, 128]
                z_sizes = {e: e - s for s, e in
                           zip([0] + z_ends[:-1], z_ends)}
                for t in range(1, T):
                    slot = Iv[:, :, t - 1:t]
                    if "v" not in ablate:
                        nc.vector.scalar_tensor_tensor(
                            slot, Vt[:], DEC, slot,
                            mybir.AluOpType.mult, mybir.AluOpType.add)
                        nc.vector.scalar_tensor_tensor(
                            Vt[:], slot, TH, slot,
                            mybir.AluOpType.is_le, mybir.AluOpType.mult)

                    if "z" not in ablate and (t + 1) in z_sizes:
                        ZBv = z_sizes[t + 1]
                        tb = t + 1 - ZBv
                        zst = zsp.tile([128, ZB * WPC], f32,
                                       tag="zst", name="zst")[:, :ZBv * WPC]
                        zsv = zst.rearrange("p (t w) -> p w t", w=WPC)
                        sct = scp.tile([128, ZB * WPC], f32, tag="sct", name="sct")
                        if tb == 0:
                            nc.vector.memset(zst[:, 0:WPC], 0.0)
                            sin = Iv[:, :, 0:ZBv - 1]
                            sflat = sct[:, 0:(ZBv - 1) * WPC]
                            zout_v = zsv[:, :, 1:ZBv]
                        else:
                            sin = Iv[:, :, tb - 1:tb + ZBv - 1]
                            sflat = sct[:, 0:ZBv * WPC]
                            zout_v = zsv
                        nc.scalar.activation(
                            sflat, sin, mybir.ActivationFunctionType.Sign,
                            bias=-TH, scale=1.0)
                        nc.scalar.activation(
                            zout_v, sflat, mybir.ActivationFunctionType.Relu)
                        nc.sync.dma_start(
                            zo[tb:tb + ZBv].rearrange("t p w -> p t w"),
                            zst.rearrange("p (t w) -> p t w", w=WPC))


def kernel(x, kernel):
    x = np.asarray(x, dtype=np.float32)
    k = np.asarray(kernel, dtype=np.float32)[0, 0]   # [3,3]
    Tn, _, H, W = x.shape
    assert (Tn, H, W) == (T, 512, 512)

    s_i = float(k[1, 0])                  # left tap = 0.15
    k_up, k_ctr, k_down, k_right = (float(k[0, 1]), float(k[1, 1]),
                                    float(k[2, 1]), float(k[1, 2]))

    nc = _build_cached(s_i, k_up, k_ctr, k_down, k_right)

    # vertical tridiagonal stationary matrix (lhsT[p, po]): u/ctr/d taps / s_i
    wvm = np.zeros((RPC, RPC), np.float32)
    cu, cc, cd = np.float32(k_up / s_i), np.float32(k_ctr / s_i), np.float32(k_down / s_i)
    for i in range(RPC):
        wvm[i, i] = cc
        if i + 1 < RPC:
            wvm[i, i + 1] = cu        # input row i feeds output row i+1's up-tap
            wvm[i + 1, i] = cd        # input row i+1 feeds output row i's down-tap
    # check: out[po] = sum_p wvm[p, po] * x[p] = cu*x[po-1] + cc*x[po] + cd*x[po+1]

    xp = np.pad(x[:, 0], ((0, 0), (1, 1), (1, 1)))   # [T, 514, 514]
    in_maps = []
    for c in range(8):
        a, b = divmod(c, 2)
        r0, w0 = 128 * a, 256 * b
        xm = np.ascontiguousarray(xp[:, 1 + r0:1 + r0 + RPC, w0:w0 + WPC + 2])
        top = xp[:, r0, w0 + 1:w0 + 1 + WPC + 2 - 2]        # halo row above, real cols
        bot = xp[:, 1 + r0 + RPC, w0 + 1:w0 + 1 + WPC]
        # xh layout [2, WPC+2, T]; only cols 1..WPC+1 are read (offset 1+w0-w0)
        xh = np.zeros((2, WPC + 2, T), np.float32)
        xh[0, 1:1 + WPC, :] = (top * np.float32(k_up / s_i)).T
        xh[1, 1:1 + WPC, :] = (bot * np.float32(k_down / s_i)).T
        in_maps.append({"xm": xm, "xh": np.ascontiguousarray(xh), "wv": wvm})

    global _LAST_IN_MAPS
    _LAST_IN_MAPS = in_maps
    res = run_bass_kernel_spmd(nc, in_maps, core_ids=list(range(8)))

    out = np.zeros((T, 1, H, W), np.float32)
    for c in range(8):
        a, b = divmod(c, 2)
        out[:, 0, 128 * a:128 * a + RPC, 256 * b:256 * b + WPC] = res.results[c]["zo"]
    return out


# revision 23
# speedup vs baseline: 1.0053x; 1.0053x over previous
"""Trainium2 Bass kernel for nn_Blur1: 3x3 cross blur + LIF neuron scan.

Reference semantics (per timestep t, state v/i per pixel):
    c    = conv2d_same(x[t], K)        # K = cross kernel (0.15 sides, 0.4 ctr)
    v_d  = 0.8*v + 0.2*i
    z[t] = (v_d - 1) > 0
    v    = (1-z)*v_d
    i    = 0.8*i + c

Strategy (8 NeuronCores = 4 H-shards x 2 W-shards, no collectives; halos are
baked into the per-core input slices on the host):
  * Scaled variables remove all per-step scalar multiplies except 0.8:
      I' = i/s_i, V' = v/(0.2*s_i), with s_i = K_left (0.15).
      c' = c/s_i = u + d + l + r + (8/3)x  (for the given cross kernel)
      V'_dec = 0.8 V' + I';  z = V'_dec > TH (TH = 1/(0.2*s_i));
      V' = (V'_dec <= TH) * V'_dec;  I' = 0.8 I' + c'
    Spike output z is bit-identical (validated in fp32 numpy vs jax ref).
  * Per core: 128 rows on the 128 SBUF partitions, 256 local W cols, T=128.
  * Conv: vertical taps (u + (8/3)c + d) via one fp32 PE matmul with a
    tridiagonal stationary matrix (PE fp32 is exact; fp32r is TF32-like and
    NOT usable). Horizontal taps (l+r) on GPSIMD. H-halo rows added via
    SWDGE DMA-accumulate directly from DRAM. All summed into hsum in SBUF.
  * Synaptic current I': one DVE tensor_tensor_scan per (8w x 128t) slice,
    with a 0.8-multiplier tile whose t=0 slots are 0.0 (per-pixel reset),
    so one scan instruction handles 8 independent pixel recurrences.
  * Membrane V': 127 sequential steps of two scalar_tensor_tensor ops on
    [128, 256]; V_dec overwrites the consumed I' slot in place.
  * Spikes: batched ACT sign -> relu over the stored V_dec values.
"""
import sys

for _p in ("/opt/trn_rl_repo",):
    if _p not in sys.path:
        sys.path.insert(0, _p)

import numpy as np
from concourse import bacc, mybir
import concourse.tile as tile
from concourse.bass_utils import run_bass_kernel_spmd

f32 = mybir.dt.float32

T = 128          # timesteps
RPC = 128        # rows per core (H=512 / 4)
WPC = 256        # cols per core (W=512 / 2)
NWC = 4          # w-chunks per core
WC = WPC // NWC  # 64 cols per chunk
NTH = 4          # t-quarters per chunk DMA
TH_T = T // NTH  # 32
SCAN_W = 16      # w-cols per scan op (F = 16*128 = 2048)
HW_W = 32        # w-cols per hsum tile
ZB = 16          # timesteps per z-output block

_CACHE = {}
_LAST_IN_MAPS = None
TUNE = {"xc_bufs": 3, "tmp_bufs": 2, "ps_bufs": 2}


def _register_const(nc, value, dtype=f32):
    t = nc.alloc_sbuf_tensor(f"const-user-{value}", [128, 1], dtype)
    nc.gpsimd.memset(t.ap(), value)
    nc.const_aps.aps[(dtype, value)] = t.ap()


def _build_cached(s_i, k_up, k_ctr, k_down, k_right):
    key = (s_i, k_up, k_ctr, k_down, k_right)
    if key not in _CACHE:
        _CACHE[key] = _build_with_consts(*key)
    return _CACHE[key]


def _build_with_consts(s_i, k_up, k_ctr, k_down, k_right):
    # activation() with a float bias needs a pre-registered const AP; patch
    # the builder to register -TH right after Bass init.
    TH = 1.0 / (0.2 * s_i)
    nc = bacc.Bacc("TRN2", target_bir_lowering=False, debug=False,
                   num_devices=8)
    _register_const(nc, -TH)
    nc.all_engine_barrier()
    _build_body(nc, s_i, k_up, k_ctr, k_down, k_right)
    if not nc.is_finalized():
        nc.finalize()
    return nc


def _build_body(nc, s_i, k_up, k_ctr, k_down, k_right, ablate=()):
    # identical to _build()'s body after nc creation
    DEC = 0.8
    TH = 1.0 / (0.2 * s_i)

    xm = nc.declare_dram_parameter("xm", [T, RPC, WPC + 2], f32, isOutput=False)
    xh = nc.declare_dram_parameter("xh", [2, WPC + 2, T], f32, isOutput=False)
    wv = nc.declare_dram_parameter("wv", [RPC, RPC], f32, isOutput=False)
    zo = nc.declare_dram_parameter("zo", [T, RPC, WPC], f32, isOutput=True)

    with tile.TileContext(nc) as tc:
        with tc.tile_pool(name="keep", bufs=1) as keep:
            wvt = keep.tile([RPC, RPC], f32)
            nc.scalar.dma_start(wvt[:], wv[:])

            It = keep.tile([128, WPC * T], f32)
            Iv = It[:].rearrange("p (w t) -> p w t", t=T)

            Vt = keep.tile([128, WPC], f32)
            nc.gpsimd.memset(Vt[:], 0.0)

            d0 = keep.tile([128, SCAN_W * T], f32)
            nc.vector.memset(d0[:], DEC)
            d0v = d0[:].rearrange("p (w t) -> p w t", t=T)
            nc.vector.memset(d0v[:, :, 0:1], 0.0)

            TBS = [1, 1, 2, 4, 8] + [16] * 7   # t-block sizes (pipeline priming)
            with tc.tile_pool(name="xc", bufs=TUNE["xc_bufs"]) as xcp, \
                 tc.tile_pool(name="tmp", bufs=TUNE["tmp_bufs"]) as tmpp, \
                 tc.tile_pool(name="ps", bufs=TUNE["ps_bufs"], space="PSUM") as psp:
                dma_engines = [nc.sync, nc.scalar]
                # c-prime accumulates directly in the I buffer (Iv views).
                t0 = 0
                for tb, TB in enumerate(TBS):
                    xc = xcp.tile([128, max(TBS) * (WPC + 2)], f32,
                                  tag="xc", name="xc")[:, :TB * (WPC + 2)]
                    xcv = xc.rearrange("p (t w) -> p t w", w=WPC + 2)
                    dma_engines[tb % 2].dma_start(
                        xcv,
                        xm[t0:t0 + TB, :, :].rearrange("t p w -> p t w"))

                    for s in range(8 if "hsum" not in ablate else 0):
                        ws = s * 32   # local w of this 32-col slice
                        nc.gpsimd.tensor_tensor(
                            Iv[:, ws:ws + 32, t0:t0 + TB],
                            xcv[:, :, ws:ws + 32].rearrange("p t w -> p w t"),
                            xcv[:, :, ws + 2:ws + 34].rearrange("p t w -> p w t"),
                            mybir.AluOpType.add)

                    for q in range(2 if "pe" not in ablate else 0):
                        wq = q * 128
                        pst = psp.tile([128, 2048], f32, tag="pst", name="pst")
                        for m in range(4):
                            wg = wq + m * 32
                            nc.tensor.matmul(
                                pst[:, m * 512:m * 512 + 32 * TB],
                                wvt[:],
                                xcv[:, :, 1 + wg:33 + wg]
                                   .rearrange("p t w -> p w t"),
                                start=True, stop=True)
                        nc.vector.tensor_tensor(
                            Iv[:, wq:wq + 128, t0:t0 + TB],
                            Iv[:, wq:wq + 128, t0:t0 + TB],
                            pst[:].rearrange("p (m c) -> p m c", m=4)
                                [:, :, :32 * TB]
                                .rearrange("p m (w t) -> p m w t", t=TB),
                            mybir.AluOpType.add)
                    t0 += TB

                # H-halo rows into partitions 0 / 127, then scan + copy-back
                for s in range(8):
                    ws = s * 32
                    nc.gpsimd.dma_start(
                        Iv[0:1, ws:ws + 32, :], xh[0:1, 1 + ws:1 + ws + 32, :],
                        accum_op=mybir.AluOpType.add)
                    nc.gpsimd.dma_start(
                        Iv[127:128, ws:ws + 32, :], xh[1:2, 1 + ws:1 + ws + 32, :],
                        accum_op=mybir.AluOpType.add)
                    for k in range(2 if "scan" not in ablate else 0):
                        lo = (ws + k * SCAN_W) * T
                        hi = (ws + (k + 1) * SCAN_W) * T
                        tmp = tmpp.tile([128, SCAN_W * T], f32,
                                        tag="tmp", name="tmp")
                        nc.vector.tensor_tensor_scan(
                            tmp[:], d0[:], It[:, lo:hi],
                            0.0, mybir.AluOpType.mult, mybir.AluOpType.add)
                        nc.scalar.copy(It[:, lo:hi], tmp[:])

            with tc.tile_pool(name="zs", bufs=2) as zsp, \
                 tc.tile_pool(name="sc", bufs=2) as scp:
                z_ends = [16, 32, 48, 64, 80, 96, 112, 120, 124, 126, 128]
                z_sizes = {e: e - s for s, e in
                           zip([0] + z_ends[:-1], z_ends)}
                for t in range(1, T):
                    slot = Iv[:, :, t - 1:t]
                    if "v" not in ablate:
                        nc.vector.scalar_tensor_tensor(
                            slot, Vt[:], DEC, slot,
                            mybir.AluOpType.mult, mybir.AluOpType.add)
                        nc.vector.scalar_tensor_tensor(
                            Vt[:], slot, TH, slot,
                            mybir.AluOpType.is_le, mybir.AluOpType.mult)

                    if "z" not in ablate and (t + 1) in z_sizes:
                        ZBv = z_sizes[t + 1]
                        tb = t + 1 - ZBv
                        zst = zsp.tile([128, ZB * WPC], f32,
                                       tag="zst", name="zst")[:, :ZBv * WPC]
                        zsv = zst.rearrange("p (t w) -> p w t", w=WPC)
                        sct = scp.tile([128, ZB * WPC], f32, tag="sct", name="sct")
                        if tb == 0:
                            nc.gpsimd.memset(zst[:, 0:WPC], 0.0)
                            sin = Iv[:, :, 0:ZBv - 1]
                            sflat = sct[:, 0:(ZBv - 1) * WPC]
                            zout_v = zsv[:, :, 1:ZBv]
                        else:
                            sin = Iv[:, :, tb - 1:tb + ZBv - 1]
                            sflat = sct[:, 0:ZBv * WPC]
                            zout_v = zsv
                        nc.scalar.activation(
                            sflat, sin, mybir.ActivationFunctionType.Sign,
                            bias=-TH, scale=1.0)
                        nc.scalar.activation(
                            zout_v, sflat, mybir.ActivationFunctionType.Relu)
                        nc.sync.dma_start(
                            zo[tb:tb + ZBv].rearrange("t p w -> p t w"),
                            zst.rearrange("p (t w) -> p t w", w=WPC))


def kernel(x, kernel):
    x = np.asarray(x, dtype=np.float32)
    k = np.asarray(kernel, dtype=np.float32)[0, 0]   # [3,3]
    Tn, _, H, W = x.shape
    assert (Tn, H, W) == (T, 512, 512)

    s_i = float(k[1, 0])                  # left tap = 0.15
    k_up, k_ctr, k_down, k_right = (float(k[0, 1]), float(k[1, 1]),
                                    float(k[2, 1]), float(k[1, 2]))

    nc = _build_cached(s_i, k_up, k_ctr, k_down, k_right)

    # vertical tridiagonal stationary matrix (lhsT[p, po]): u/ctr/d taps / s_i
    wvm = np.zeros((RPC, RPC), np.float32)
    cu, cc, cd = np.float32(k_up / s_i), np.float32(k_ctr / s_i), np.float32(k_down / s_i)
    for i in range(RPC):
        wvm[i, i] = cc
        if i + 1 < RPC:
            wvm[i, i + 1] = cu        # input row i feeds output row i+1's up-tap
            wvm[i + 1, i] = cd        # input row i+1 feeds output row i's down-tap
    # check: out[po] = sum_p wvm[p, po] * x[p] = cu*x[po-1] + cc*x[po] + cd*x[po+1]

    xp = np.pad(x[:, 0], ((0, 0), (1, 1), (1, 1)))   # [T, 514, 514]
    in_maps = []
    for c in range(8):
        a, b = divmod(c, 2)
        r0, w0 = 128 * a, 256 * b
        xm = np.ascontiguousarray(xp[:, 1 + r0:1 + r0 + RPC, w0:w0 + WPC + 2])
        top = xp[:, r0, w0 + 1:w0 + 1 + WPC + 2 - 2]        # halo row above, real cols
        bot = xp[:, 1 + r0 + RPC, w0 + 1:w0 + 1 + WPC]
        # xh layout [2, WPC+2, T]; only cols 1..WPC+1 are read (offset 1+w0-w0)
        xh = np.zeros((2, WPC + 2, T), np.float32)
        xh[0, 1:1 + WPC, :] = (top * np.float32(k_up / s_i)).T
        xh[1, 1:1 + WPC, :] = (bot * np.float32(k_down / s_i)).T
        in_maps.append({"xm": xm, "xh": np.ascontiguousarray(xh), "wv": wvm})

    global _LAST_IN_MAPS
    _LAST_IN_MAPS = in_maps
    res = run_bass_kernel_spmd(nc, in_maps, core_ids=list(range(8)))

    out = np.zeros((T, 1, H, W), np.float32)
    for c in range(8):
        a, b = divmod(c, 2)
        out[:, 0, 128 * a:128 * a + RPC, 256 * b:256 * b + WPC] = res.results[c]["zo"]
    return out
